# revision 1
# baseline (speedup 1.0000x reference)
"""LoRA linear kernel for Trainium2 (8 NeuronCores, SPMD data-parallel).

Computes out = x @ (A @ B) for
    x: [4, 2048, 4096] f32, A: [4096, 16] f32, B: [16, 4096] f32
by reassociating to (x @ A) @ B  (4.3 GFLOP instead of 274 GFLOP).

Sharding: x is split row-wise (batch*seq = 8192 rows -> 1024 rows/core).
Each core's shard is pre-transposed on the host to xT [4096, 1024] so the
contraction dim (d_in) lands on SBUF partitions naturally:

  stage 1:  tT[16, n]   = sum_c A_c[128,16].T @ xT_c[128, n]   (PSUM accum)
  stage 2:  out[128, d] = tT[:, rb].T @ B[16, d]               (single MM)

A and B are replicated to every core. No collectives.
"""

import numpy as np

import concourse.bass as bass
import concourse.bacc as bacc
import concourse.mybir as mybir
from concourse.tile import TileContext
from concourse.bass_utils import run_bass_kernel_spmd

N_CORES = 8
BATCH, SEQ, D_IN, D_OUT, R = 4, 2048, 4096, 4096, 16
ROWS = BATCH * SEQ              # 8192
RPC = ROWS // N_CORES           # 1024 rows per core
KC = D_IN // 128                # 32 contraction chunks of 128
RCHUNK = 256                    # rows per stage-1 chunk
NCH = RPC // RCHUNK             # 4 chunks per core
DC = 512                        # d_out columns per stage-2 matmul (PSUM bank)
NDC = D_OUT // DC               # 8

F32 = mybir.dt.float32

_cache = {}


def _build_packed(mm_dtype=F32, reps=1, loop_n=0, nway=2, mm2=None,
                  split_copy=False):
    """Packed variant: stage-1 col-tiling + stage-2 row-tiling via
    tile_position, processing `nway` 128-row blocks concurrently in
    disjoint 32-partition strips of the PE array."""
    nc = bacc.Bacc("TRN2", target_bir_lowering=False)
    rchunk = 128 * nway
    nch = RPC // rchunk
    xT = nc.dram_tensor("xT", [D_IN, RPC], mm_dtype, kind="ExternalInput")
    A = nc.dram_tensor("A", [D_IN, R], mm_dtype, kind="ExternalInput")
    Bw = nc.dram_tensor("Bw", [R, D_OUT], mm_dtype, kind="ExternalInput")
    out = nc.dram_tensor("out", [RPC, D_OUT], F32, kind="ExternalOutput")

    xT3 = xT.rearrange("(c p) n -> p c n", p=128)   # [128, KC, RPC]
    A3 = A.rearrange("(c p) r -> p c r", p=128)     # [128, KC, R]

    with TileContext(nc) as tc:
        with (
            tc.tile_pool(name="consts", bufs=1) as cpool,
            tc.tile_pool(name="xin", bufs=(2 if nway <= 2 else 3)) as xpool,
            tc.tile_pool(name="tbuf", bufs=2) as tpool,
            tc.tile_pool(name="obuf", bufs=min(2 * nway, 4)) as opool,
            tc.tile_pool(name="pt", bufs=2, space="PSUM") as ptpool,
            tc.tile_pool(name="po", bufs=min(2 * nway, 6), space="PSUM") as popool,
        ):
            a_tile = cpool.tile([128, KC, R], mm_dtype)
            nc.sync.dma_start(out=a_tile[:], in_=A3[:, :, :])
            # B replicated into partition strips 32g..32g+16
            b_dtype = mm2 if mm2 is not None else mm_dtype
            b4 = cpool.tile([128, D_OUT], b_dtype)
            for g in range(nway):
                dma = nc.gpsimd if b_dtype != mm_dtype else nc.sync
                dma.dma_start(out=b4[32 * g:32 * g + R, :], in_=Bw[:, :])

            nsplit = 2 if nway > 2 else 1
            kcs = KC // nsplit

            def body():
                for rc in range(nch * reps):
                    rc = rc % nch
                    n0 = rc * rchunk

                    # stage 1: nway concurrent col-strip matmuls;
                    # strip g accumulates tT of row-block g into
                    # psum partitions 32g..32g+16.
                    pt = ptpool.tile([128, 128], F32)
                    for h in range(nsplit):
                        xt = xpool.tile([128, kcs, rchunk], mm_dtype,
                                        name="xt", tag="xt")
                        nc.sync.dma_start(
                            out=xt[:],
                            in_=xT3[:, h * kcs:(h + 1) * kcs, n0:n0 + rchunk])
                        for c in range(kcs):
                            for g in range(nway):
                                nc.tensor.matmul(
                                    pt[32 * g:32 * g + R, :],
                                    a_tile[:, h * kcs + c, :],
                                    xt[:, c, 128 * g:128 * (g + 1)],
                                    start=(h == 0 and c == 0),
                                    stop=(h == nsplit - 1 and c == kcs - 1),
                                    tile_position=(0, 32 * g),
                                    skip_group_check=True,
                                )
                    tT4 = tpool.tile([128, 128],
                                     mm2 if mm2 is not None else mm_dtype)
                    nc.vector.tensor_copy(tT4[:], pt[:])

                    # stage 2: nway concurrent row-strip matmuls
                    osbs = [opool.tile([128, D_OUT], F32, name=f"osb{g}",
                                       tag="osb")
                            for g in range(nway)]
                    for dc in range(NDC):
                        for g in range(nway):
                            po = popool.tile([128, DC], F32, name=f"po{g}",
                                             tag="po")
                            lhsT = tT4[32 * g:32 * g + R, :]
                            rhs = b4[32 * g:32 * g + R, dc * DC:(dc + 1) * DC]
                            nc.tensor.matmul(
                                po[:],
                                lhsT,
                                rhs,
                                start=True,
                                stop=True,
                                tile_position=(32 * g, 0),
                                skip_group_check=True,
                            )
                            if split_copy and dc % 3 == 2:
                                nc.scalar.copy(
                                    out=osbs[g][:, dc * DC:(dc + 1) * DC],
                                    in_=po[:])
                            else:
                                nc.vector.tensor_copy(
                                    osbs[g][:, dc * DC:(dc + 1) * DC], po[:])
                    for g in range(nway):
                        row0 = n0 + 128 * g
                        nc.sync.dma_start(out=out[row0:row0 + 128, :],
                                          in_=osbs[g][:])

            if loop_n:
                with tc.For_i(0, loop_n, 1):
                    body()
            else:
                body()
    nc.compile()
    return nc


def _build(mm_dtype=F32, reps=1, loop_n=0):
    nc = bacc.Bacc("TRN2", target_bir_lowering=False)
    xT = nc.dram_tensor("xT", [D_IN, RPC], mm_dtype, kind="ExternalInput")
    A = nc.dram_tensor("A", [D_IN, R], mm_dtype, kind="ExternalInput")
    Bw = nc.dram_tensor("Bw", [R, D_OUT], mm_dtype, kind="ExternalInput")
    out = nc.dram_tensor("out", [RPC, D_OUT], F32, kind="ExternalOutput")

    xT3 = xT.rearrange("(c p) n -> p c n", p=128)   # [128, KC, RPC]
    A3 = A.rearrange("(c p) r -> p c r", p=128)     # [128, KC, R]

    with TileContext(nc) as tc:
        with (
            tc.tile_pool(name="consts", bufs=1) as cpool,
            tc.tile_pool(name="xin", bufs=2) as xpool,
            tc.tile_pool(name="tbuf", bufs=2) as tpool,
            tc.tile_pool(name="obuf", bufs=2) as opool,
            tc.tile_pool(name="pt", bufs=2, space="PSUM") as ptpool,
            tc.tile_pool(name="po", bufs=4, space="PSUM") as popool,
        ):
            a_tile = cpool.tile([128, KC, R], mm_dtype)
            nc.sync.dma_start(out=a_tile[:], in_=A3[:, :, :])
            b_tile = cpool.tile([R, D_OUT], mm_dtype)
            nc.sync.dma_start(out=b_tile[:], in_=Bw[:, :])

            def body():
                for rc in range(NCH * reps):
                    rc = rc % NCH
                    n0 = rc * RCHUNK
                    xt = xpool.tile([128, KC, RCHUNK], mm_dtype)
                    nc.sync.dma_start(out=xt[:], in_=xT3[:, :, n0:n0 + RCHUNK])

                    # stage 1: tT [16, RCHUNK] = (x_chunk @ A).T
                    pt = ptpool.tile([R, RCHUNK], F32)
                    for c in range(KC):
                        nc.tensor.matmul(
                            pt[:],
                            a_tile[:, c, :],
                            xt[:, c, :],
                            start=(c == 0),
                            stop=(c == KC - 1),
                        )
                    tT = tpool.tile([R, RCHUNK], mm_dtype)
                    nc.vector.tensor_copy(tT[:], pt[:])

                    # stage 2: out rows = tT.T @ B, one 128-row block at a time
                    for rb in range(RCHUNK // 128):
                        osb = opool.tile([128, D_OUT], F32)
                        for dc in range(NDC):
                            po = popool.tile([128, DC], F32)
                            nc.tensor.matmul(
                                po[:],
                                tT[:, rb * 128:(rb + 1) * 128],
                                b_tile[:, dc * DC:(dc + 1) * DC],
                                start=True,
                                stop=True,
                            )
                            nc.vector.tensor_copy(
                                osb[:, dc * DC:(dc + 1) * DC], po[:])
                        row0 = n0 + rb * 128
                        nc.sync.dma_start(out=out[row0:row0 + 128, :], in_=osb[:])

            if loop_n:
                with tc.For_i(0, loop_n, 1):
                    body()
            else:
                body()
    nc.compile()
    return nc


def _get_nc(mm_dtype=F32, reps=1, loop_n=0, layout="simple", nway=2, mm2=None,
            split_copy=False):
    key = (str(mm_dtype), reps, loop_n, layout, nway, str(mm2), split_copy)
    if key not in _cache:
        if layout == "packed":
            _cache[key] = _build_packed(mm_dtype, reps, loop_n, nway, mm2,
                                        split_copy)
        else:
            _cache[key] = _build(mm_dtype, reps, loop_n)
    return _cache[key]


def kernel(x, A, B, trace=False, mm_dtype=F32):
    x = np.asarray(x, dtype=np.float32)
    A = np.ascontiguousarray(np.asarray(A, dtype=np.float32))
    B = np.ascontiguousarray(np.asarray(B, dtype=np.float32))
    xf = x.reshape(ROWS, D_IN)

    nc = _get_nc(mm_dtype)
    in_maps = []
    for i in range(N_CORES):
        xs = xf[i * RPC:(i + 1) * RPC]                 # [1024, 4096]
        xT = np.ascontiguousarray(xs.T)                # [4096, 1024]
        in_maps.append({"xT": xT, "A": A, "Bw": B})

    res = run_bass_kernel_spmd(nc, in_maps, list(range(N_CORES)), trace=trace)
    outs = [res.results[i]["out"] for i in range(N_CORES)]
    full = np.concatenate(outs, axis=0).reshape(BATCH, SEQ, D_OUT)
    if trace:
        kernel.last_exec_time_ns = res.exec_time_ns
        kernel.last_results = res
    return full



# revision 2
# speedup vs baseline: 2.9948x; 2.9948x over previous
"""LoRA linear kernel for Trainium2 (8 NeuronCores, SPMD data-parallel).

Computes out = x @ (A @ B) for
    x: [4, 2048, 4096] f32, A: [4096, 16] f32, B: [16, 4096] f32
by reassociating to (x @ A) @ B  (4.3 GFLOP instead of 274 GFLOP).

Sharding: x is split row-wise (batch*seq = 8192 rows -> 1024 rows/core).
A and B are replicated to every core. No collectives.

All matmul operands are fp16 (1 cycle/row on the PE array vs 4 for
fp32) and the output is shipped back as fp16 and upcast on the host,
halving HBM traffic in both directions. PSUM accumulation stays fp32.
Host-side prep lays x out as xTb[p, rc, c, n] so each row-chunk's DMA
is a single per-partition-contiguous 16 KiB line.

  stage 1:  tT[16, n]   = sum_c A_c[128,16].T @ xTb_c[128, n]  (PSUM accum)
  stage 2:  out[128, d] = tT[:, rb].T @ B[16, d]               (single MM)
"""

import numpy as np

import concourse.bass as bass
import concourse.bacc as bacc
import concourse.mybir as mybir
from concourse.tile import TileContext
from concourse.bass_utils import run_bass_kernel_spmd

N_CORES = 8
BATCH, SEQ, D_IN, D_OUT, R = 4, 2048, 4096, 4096, 16
ROWS = BATCH * SEQ              # 8192
RPC = ROWS // N_CORES           # 1024 rows per core
KC = D_IN // 128                # 32 contraction chunks of 128
RCHUNK = 256                    # rows per stage-1 chunk
NCH = RPC // RCHUNK             # 4 chunks per core
DC = 512                        # d_out columns per stage-2 matmul (PSUM bank)
NDC = D_OUT // DC               # 8

F32 = mybir.dt.float32
F16 = mybir.dt.float16

_cache = {}


def _build(mm_dtype=F16):
    nc = bacc.Bacc("TRN2", target_bir_lowering=False)
    # xTb[p, rc, c, n] = x_shard[rc*RCHUNK + n, c*128 + p]
    xTb = nc.dram_tensor("xTb", [128, NCH, KC, RCHUNK], mm_dtype,
                         kind="ExternalInput")
    A = nc.dram_tensor("A", [D_IN, R], mm_dtype, kind="ExternalInput")
    Bw = nc.dram_tensor("Bw", [R, D_OUT], mm_dtype, kind="ExternalInput")
    out = nc.dram_tensor("out", [RPC, D_OUT], mm_dtype,
                         kind="ExternalOutput")

    A3 = A.rearrange("(c p) r -> p c r", p=128)     # [128, KC, R]

    with TileContext(nc) as tc:
        with (
            tc.tile_pool(name="consts", bufs=1) as cpool,
            tc.tile_pool(name="xin", bufs=3) as xpool,
            tc.tile_pool(name="tbuf", bufs=2) as tpool,
            tc.tile_pool(name="obuf", bufs=3) as opool,
            tc.tile_pool(name="pt", bufs=2, space="PSUM") as ptpool,
            tc.tile_pool(name="po", bufs=4, space="PSUM") as popool,
        ):
            a_tile = cpool.tile([128, KC, R], mm_dtype)
            nc.sync.dma_start(out=a_tile[:], in_=A3[:, :, :])
            b_tile = cpool.tile([R, D_OUT], mm_dtype)
            nc.sync.dma_start(out=b_tile[:], in_=Bw[:, :])

            for rc in range(NCH):
                xt = xpool.tile([128, KC, RCHUNK], mm_dtype)
                nc.sync.dma_start(out=xt[:], in_=xTb[:, rc, :, :])

                # stage 1: tT [16, RCHUNK] = (x_chunk @ A).T
                pt = ptpool.tile([R, RCHUNK], F32)
                for c in range(KC):
                    nc.tensor.matmul(
                        pt[:],
                        a_tile[:, c, :],
                        xt[:, c, :],
                        start=(c == 0),
                        stop=(c == KC - 1),
                    )
                tT = tpool.tile([R, RCHUNK], mm_dtype)
                nc.vector.tensor_copy(tT[:], pt[:])

                # stage 2: out rows = tT.T @ B, one 128-row block at a time
                for rb in range(RCHUNK // 128):
                    osb = opool.tile([128, D_OUT], mm_dtype)
                    for dc in range(NDC):
                        po = popool.tile([128, DC], F32)
                        nc.tensor.matmul(
                            po[:],
                            tT[:, rb * 128:(rb + 1) * 128],
                            b_tile[:, dc * DC:(dc + 1) * DC],
                            start=True,
                            stop=True,
                        )
                        if dc % 3 == 2:
                            nc.scalar.copy(
                                out=osb[:, dc * DC:(dc + 1) * DC], in_=po[:])
                        else:
                            nc.vector.tensor_copy(
                                osb[:, dc * DC:(dc + 1) * DC], po[:])
                    row0 = rc * RCHUNK + rb * 128
                    nc.sync.dma_start(out=out[row0:row0 + 128, :],
                                      in_=osb[:])
    nc.compile()
    return nc


def _get_nc(mm_dtype=F16):
    key = (str(mm_dtype),)
    if key not in _cache:
        _cache[key] = _build(mm_dtype)
    return _cache[key]


def _np_dtype(mm_dtype):
    return {str(F16): np.float16, str(mybir.dt.bfloat16): "bfloat16"}.get(
        str(mm_dtype), np.float16)


def kernel(x, A, B, trace=False, mm_dtype=None):
    if mm_dtype is None:
        mm_dtype = F16
    npdt = _np_dtype(mm_dtype)
    x = np.asarray(x, dtype=np.float32)
    Ah = np.ascontiguousarray(np.asarray(A)).astype(npdt)
    Bh = np.ascontiguousarray(np.asarray(B)).astype(npdt)
    xf = x.reshape(ROWS, D_IN)

    nc = _get_nc(mm_dtype)
    in_maps = []
    for i in range(N_CORES):
        xs = xf[i * RPC:(i + 1) * RPC]                 # [1024, 4096]
        # xTb[p, rc, c, n] = xs[rc*RCHUNK+n, c*128+p]
        xTb = np.ascontiguousarray(
            xs.reshape(NCH, RCHUNK, KC, 128).transpose(3, 0, 2, 1)
        ).astype(npdt)
        in_maps.append({"xTb": xTb, "A": Ah, "Bw": Bh})

    res = run_bass_kernel_spmd(nc, in_maps, list(range(N_CORES)), trace=trace)
    outs = [res.results[i]["out"] for i in range(N_CORES)]
    full = np.concatenate(outs, axis=0).astype(np.float32)
    full = full.reshape(BATCH, SEQ, D_OUT)
    if trace:
        kernel.last_exec_time_ns = res.exec_time_ns
        kernel.last_results = res
    return full


# revision 3
# speedup vs baseline: 3.2383x; 1.0813x over previous
"""LoRA linear kernel for Trainium2 (8 NeuronCores, SPMD data-parallel).

Computes out = x @ (A @ B) for
    x: [4, 2048, 4096] f32, A: [4096, 16] f32, B: [16, 4096] f32
by reassociating to (x @ A) @ B  (4.3 GFLOP instead of 274 GFLOP).

Sharding: x is split row-wise (batch*seq = 8192 rows -> 1024 rows/core).
A and B are replicated to every core. No collectives.

All matmul operands are fp16 (1 cycle/row on the PE array vs 4 for
fp32) and the output is shipped back as fp16 and upcast on the host,
halving HBM traffic in both directions. PSUM accumulation stays fp32.
Host-side prep lays x out as xTb[p, rc, c, n] so each row-chunk's DMA
is a single per-partition-contiguous line.

The R=16 contraction/output dims would leave 7/8 of the PE array idle,
so NWAY=4 row-blocks are processed concurrently in disjoint 32-wide
strips of the array via tile_position:
  stage 1 (col strips): strip g computes tT_g[16,128] = (x_blk_g @ A).T
      accumulating into PSUM partitions 32g..32g+16.
  stage 2 (row strips): strip g computes out_blk_g[128, dc] =
      tT_g.T @ B from SBUF partitions 32g..32g+16 (B replicated there).
"""

import numpy as np

import concourse.bass as bass
import concourse.bacc as bacc
import concourse.mybir as mybir
from concourse.tile import TileContext
from concourse.bass_utils import run_bass_kernel_spmd

N_CORES = 8
BATCH, SEQ, D_IN, D_OUT, R = 4, 2048, 4096, 4096, 16
ROWS = BATCH * SEQ              # 8192
RPC = ROWS // N_CORES           # 1024 rows per core
KC = D_IN // 128                # 32 contraction chunks of 128
DC = 512                        # d_out columns per stage-2 matmul (PSUM bank)
NDC = D_OUT // DC               # 8

F32 = mybir.dt.float32
F16 = mybir.dt.float16

NWAY = 4                        # concurrent 128-row blocks (PE strips)
RCHUNK = 128 * NWAY             # 512 rows per chunk
NCH = RPC // RCHUNK             # 2 chunks per core
NSPLIT = 2                      # split each chunk's input DMA over KC

_cache = {}


def _build(mm_dtype=F16):
    nc = bacc.Bacc("TRN2", target_bir_lowering=False)
    # xTb[p, rc, c, n] = x_shard[rc*RCHUNK + n, c*128 + p]
    xTb = nc.dram_tensor("xTb", [128, NCH, KC, RCHUNK], mm_dtype,
                         kind="ExternalInput")
    A = nc.dram_tensor("A", [D_IN, R], mm_dtype, kind="ExternalInput")
    Bw = nc.dram_tensor("Bw", [R, D_OUT], mm_dtype, kind="ExternalInput")
    out = nc.dram_tensor("out", [RPC, D_OUT], mm_dtype,
                         kind="ExternalOutput")

    A3 = A.rearrange("(c p) r -> p c r", p=128)     # [128, KC, R]
    kcs = KC // NSPLIT

    with TileContext(nc) as tc:
        with (
            tc.tile_pool(name="consts", bufs=1) as cpool,
            tc.tile_pool(name="xin", bufs=3) as xpool,
            tc.tile_pool(name="tbuf", bufs=2) as tpool,
            tc.tile_pool(name="obuf", bufs=2 * NWAY) as opool,
            tc.tile_pool(name="pt", bufs=2, space="PSUM") as ptpool,
            tc.tile_pool(name="po", bufs=6, space="PSUM") as popool,
        ):
            a_tile = cpool.tile([128, KC, R], mm_dtype)
            nc.sync.dma_start(out=a_tile[:], in_=A3[:, :, :])
            # B replicated into partition strips 32g..32g+16
            b4 = cpool.tile([128, D_OUT], mm_dtype)
            for g in range(NWAY):
                nc.sync.dma_start(out=b4[32 * g:32 * g + R, :], in_=Bw[:, :])

            for rc in range(NCH):
                n0 = rc * RCHUNK

                # stage 1: NWAY concurrent col-strip matmuls; strip g
                # accumulates tT of row-block g into psum partitions
                # 32g..32g+16.
                pt = ptpool.tile([128, 128], F32)
                for h in range(NSPLIT):
                    xt = xpool.tile([128, kcs, RCHUNK], mm_dtype,
                                    name="xt", tag="xt")
                    nc.sync.dma_start(
                        out=xt[:],
                        in_=xTb[:, rc, h * kcs:(h + 1) * kcs, :])
                    for c in range(kcs):
                        for g in range(NWAY):
                            nc.tensor.matmul(
                                pt[32 * g:32 * g + R, :],
                                a_tile[:, h * kcs + c, :],
                                xt[:, c, 128 * g:128 * (g + 1)],
                                start=(h == 0 and c == 0),
                                stop=(h == NSPLIT - 1 and c == kcs - 1),
                                tile_position=(0, 32 * g),
                                skip_group_check=True,
                            )
                tT4 = tpool.tile([128, 128], mm_dtype)
                nc.vector.tensor_copy(tT4[:], pt[:])

                # stage 2: NWAY concurrent row-strip matmuls per dc
                osbs = [opool.tile([128, D_OUT], mm_dtype, name=f"osb{g}",
                                   tag="osb")
                        for g in range(NWAY)]
                for dc in range(NDC):
                    for g in range(NWAY):
                        po = popool.tile([128, DC], F32, name=f"po{g}",
                                         tag="po")
                        nc.tensor.matmul(
                            po[:],
                            tT4[32 * g:32 * g + R, :],
                            b4[32 * g:32 * g + R, dc * DC:(dc + 1) * DC],
                            start=True,
                            stop=True,
                            tile_position=(32 * g, 0),
                            skip_group_check=True,
                        )
                        # Split PSUM evacuation between DVE and ACT
                        if (dc * NWAY + g) % 2 == 0:
                            nc.vector.tensor_copy(
                                osbs[g][:, dc * DC:(dc + 1) * DC], po[:])
                        else:
                            nc.scalar.copy(
                                out=osbs[g][:, dc * DC:(dc + 1) * DC],
                                in_=po[:])
                for g in range(NWAY):
                    row0 = n0 + 128 * g
                    nc.sync.dma_start(out=out[row0:row0 + 128, :],
                                      in_=osbs[g][:])
    nc.compile()
    return nc


def _get_nc(mm_dtype=F16):
    key = (str(mm_dtype),)
    if key not in _cache:
        _cache[key] = _build(mm_dtype)
    return _cache[key]


def kernel(x, A, B, trace=False, mm_dtype=None):
    if mm_dtype is None:
        mm_dtype = F16
    x = np.asarray(x, dtype=np.float32)
    Ah = np.ascontiguousarray(np.asarray(A)).astype(np.float16)
    Bh = np.ascontiguousarray(np.asarray(B)).astype(np.float16)
    xf = x.reshape(ROWS, D_IN)

    nc = _get_nc(mm_dtype)
    in_maps = []
    for i in range(N_CORES):
        xs = xf[i * RPC:(i + 1) * RPC]                 # [1024, 4096]
        # xTb[p, rc, c, n] = xs[rc*RCHUNK+n, c*128+p]
        xTb = np.ascontiguousarray(
            xs.reshape(NCH, RCHUNK, KC, 128).transpose(3, 0, 2, 1)
        ).astype(np.float16)
        in_maps.append({"xTb": xTb, "A": Ah, "Bw": Bh})

    res = run_bass_kernel_spmd(nc, in_maps, list(range(N_CORES)), trace=trace)
    outs = [res.results[i]["out"] for i in range(N_CORES)]
    full = np.concatenate(outs, axis=0).astype(np.float32)
    full = full.reshape(BATCH, SEQ, D_OUT)
    if trace:
        kernel.last_exec_time_ns = res.exec_time_ns
        kernel.last_results = res
    return full
